# revision 1
# baseline (speedup 1.0000x reference)
"""Trainium2 Bass kernel for BinaryTokenClassificationModel (segment_reduce).

Reference semantics (B=16, L=2048, H=1024, W=1024):
    src = segment_mean(hidden, source_word_ids)   # [B,W,H]
    tgt = segment_mean(hidden, target_word_ids)   # [B,W,H]
    logits[b,s,t,0] = src[b,s]@w_s + tgt[b,t]@w_t + bias

Key algebraic restructuring: the pooled [B,W,H] tensors are never
materialized.  Because the classifier is linear,
    src_proj[b,s] = segment_mean_s( hidden[b,l] @ w_s )
so we compute per-token scalar dots (fused multiply+reduce on the DVE),
segment-reduce the *scalars* (via tiny one-hot matmuls on the PE, using
the factorization w = 128*q + r), and emit the [W,W] output as an outer
broadcast-sum.

Sharding: data-parallel over batch — 2 examples per NeuronCore on 8 cores.
The tiny classifier weights are replicated.
"""

from contextlib import ExitStack

import ml_dtypes
import numpy as np

import concourse.mybir as mybir
import concourse.tile as tile
from concourse import bacc
from concourse.bass_utils import run_bass_kernel_spmd
from concourse.masks import make_identity

P = 128          # partitions
B = 16           # full batch
NCORES = 8
BLOC = B // NCORES   # batches per core = 2
L = 2048         # tokens
H = 1024         # hidden
W = 1024         # words
Q = W // P       # 8 word chunks
NI = L // P      # 16 token tiles per batch (token l = p*NI + i)
ICH = 4          # token tiles loaded per DMA

F32 = mybir.dt.float32
BF16 = mybir.dt.bfloat16
I32 = mybir.dt.int32

# Compute dtype for the per-token dot products. "bf16" halves both the
# hidden-states DMA traffic and the DVE multiply cost (2x packed mode);
# reductions stay fp32 (DVE/ACT accumulate internally in fp32).
DOT_DTYPE = "bf16"
HDT = BF16 if DOT_DTYPE == "bf16" else F32
# Fraction of dot tiles reduced via the fused DVE op (affine_mul_reduce);
# the rest go DVE-mul + ACT-accumulate to balance engine load.
AMR_PATTERN = 8   # (tile_index % AMR_PATTERN) < AMR_KEEP -> fused DVE path
AMR_KEEP = 3

_CACHE = {}


def _build_module():
    nc = bacc.Bacc(None, target_bir_lowering=False, debug=False)
    names = {}
    with tile.TileContext(nc) as tc, ExitStack() as ctx:
        dram = ctx.enter_context(tc.tile_pool(name="dram", bufs=1, space="DRAM"))
        sb_c = ctx.enter_context(tc.tile_pool(name="const", bufs=1))
        sb_h = ctx.enter_context(tc.tile_pool(name="hid", bufs=4))
        sb_s = ctx.enter_context(tc.tile_pool(name="small", bufs=2))
        sb_o = ctx.enter_context(tc.tile_pool(name="outp", bufs=6))
        ps = ctx.enter_context(tc.tile_pool(name="psum", bufs=2, space="PSUM"))

        hid_d = [dram.tile([L, H], HDT, kind="ExternalInput", name=f"hid{b}")
                 for b in range(BLOC)]
        src_d = [dram.tile([L], I32, kind="ExternalInput", name=f"srcids{b}")
                 for b in range(BLOC)]
        tgt_d = [dram.tile([L], I32, kind="ExternalInput", name=f"tgtids{b}")
                 for b in range(BLOC)]
        w_d = dram.tile([P, 2 * H], HDT, kind="ExternalInput")
        b_d = dram.tile([P, 1], F32, kind="ExternalInput")
        out_d = [dram.tile([W, W], F32, kind="ExternalOutput", name=f"logits{b}")
                 for b in range(BLOC)]

        names["hid"] = [t.name for t in hid_d]
        names["src"] = [t.name for t in src_d]
        names["tgt"] = [t.name for t in tgt_d]
        names["w"] = w_d.name
        names["b"] = b_d.name
        names["out"] = [t.name for t in out_d]

        # ---- constants ----
        w_s = sb_c.tile([P, H], HDT, tag="ws")
        w_t = sb_c.tile([P, H], HDT, tag="wt")
        nc.scalar.dma_start(out=w_s[:], in_=w_d[:, 0:H])
        nc.scalar.dma_start(out=w_t[:], in_=w_d[:, H:2 * H])
        b_bc = sb_c.tile([P, 1], F32, tag="bb")
        nc.scalar.dma_start(out=b_bc[:], in_=b_d[:])

        # iota_r16[p, i, r] = r ; iota_q16[p, i, q] = q  (batched one-hot builds)
        iota_r16 = sb_c.tile([P, NI, P], F32, tag="ior")
        nc.gpsimd.iota(iota_r16[:], pattern=[[0, NI], [1, P]], base=0,
                       channel_multiplier=0, allow_small_or_imprecise_dtypes=True)
        iota_q16 = sb_c.tile([P, NI, Q], F32, tag="ioq")
        nc.gpsimd.iota(iota_q16[:], pattern=[[0, NI], [1, Q]], base=0,
                       channel_multiplier=0, allow_small_or_imprecise_dtypes=True)
        ident = sb_c.tile([P, P], F32, tag="id")
        make_identity(nc, ident[:])
        ones = sb_c.tile([P, P], F32, tag="ones")
        nc.vector.memset(ones[:], 1.0)

        for b in range(BLOC):
            hid_ap = hid_d[b][:].rearrange("(p i) h -> p i h", p=P)  # l = p*NI+i

            # ---- ids -> (q, r) one-hots, before the dot loop ----
            qf = {}
            rf = {}
            for side, ids_dram in (("s", src_d[b]), ("t", tgt_d[b])):
                ids_t = sb_s.tile([P, NI], I32, tag="ids")
                nc.sync.dma_start(out=ids_t[:],
                                  in_=ids_dram[:].rearrange("(p i) -> p i", p=P))
                q_i = sb_s.tile([P, NI], I32, tag="qi")
                r_i = sb_s.tile([P, NI], I32, tag="ri")
                nc.vector.tensor_scalar(out=q_i[:], in0=ids_t[:], scalar1=7,
                                        scalar2=None,
                                        op0=mybir.AluOpType.logical_shift_right)
                nc.vector.tensor_scalar(out=r_i[:], in0=ids_t[:], scalar1=127,
                                        scalar2=None,
                                        op0=mybir.AluOpType.bitwise_and)
                qf[side] = sb_s.tile([P, NI], F32, tag=f"qf{side}", name=f"qf{side}")
                rf[side] = sb_s.tile([P, NI], F32, tag=f"rf{side}", name=f"rf{side}")
                nc.vector.tensor_copy(out=qf[side][:], in_=q_i[:])
                nc.vector.tensor_copy(out=rf[side][:], in_=r_i[:])

            or_all = {}
            mdoq = {}
            segT = {}
            dots = {}
            for side in ("s", "t"):
                or_all[side] = sb_s.tile([P, NI, P], F32, tag=f"orall{side}",
                                         name=f"orall{side}")
                nc.vector.tensor_tensor(
                    out=or_all[side][:], in0=iota_r16[:],
                    in1=rf[side][:].to_broadcast([P, NI, P]),
                    op=mybir.AluOpType.is_equal)
                mdoq[side] = sb_s.tile([P, NI, 2 * Q], F32, tag=f"mdoq{side}",
                                       name=f"mdoq{side}")
                nc.vector.tensor_tensor(
                    out=mdoq[side][:, :, Q:2 * Q], in0=iota_q16[:],
                    in1=qf[side][:].to_broadcast([P, NI, Q]),
                    op=mybir.AluOpType.is_equal)
                segT[side] = ps.tile([2 * Q, P], F32, space="PSUM",
                                     tag=f"segT{side}", name=f"segT{side}")
                dots[side] = sb_s.tile([P, NI], F32, tag=f"dots{side}",
                                       name=f"dots{side}")

            # ---- dots (fused mul+reduce on DVE) + interleaved seg matmuls ----
            for ic in range(NI // ICH):
                ht = sb_h.tile([P, ICH, H], HDT, tag="ht")
                nc.sync.dma_start(out=ht[:], in_=hid_ap[:, ic * ICH:(ic + 1) * ICH, :])
                for k in range(ICH):
                    i = ic * ICH + k
                    for sidx, (side, wt) in enumerate((("t", w_t), ("s", w_s))):
                        scratch = sb_s.tile([P, H], HDT, tag="scr", bufs=6)
                        if HDT is F32 or (2 * i + sidx) % AMR_PATTERN < AMR_KEEP:
                            nc.vector.affine_mul_reduce(
                                out=scratch[:], accum_out=dots[side][:, i:i + 1],
                                in0=ht[:, k, :], in1=wt[:], scale=1.0, bias=0.0)
                        else:
                            nc.vector.tensor_tensor(
                                out=scratch[:], in0=ht[:, k, :], in1=wt[:],
                                op=mybir.AluOpType.mult)
                            scratch2 = sb_s.tile([P, H], HDT, tag="scr2", bufs=6)
                            nc.scalar.activation(
                                out=scratch2[:], in_=scratch[:],
                                func=mybir.ActivationFunctionType.Copy,
                                accum_out=dots[side][:, i:i + 1])
                # this chunk's md columns, then its segment matmuls
                sl = slice(ic * ICH, (ic + 1) * ICH)
                for side in ("s", "t"):
                    nc.vector.tensor_tensor(
                        out=mdoq[side][:, sl, 0:Q], in0=mdoq[side][:, sl, Q:2 * Q],
                        in1=dots[side][:, sl].to_broadcast([P, ICH, Q]),
                        op=mybir.AluOpType.mult)
                    for k in range(ICH):
                        i = ic * ICH + k
                        nc.tensor.matmul(out=segT[side][:],
                                         lhsT=mdoq[side][:, i, :],
                                         rhs=or_all[side][:, i, :],
                                         start=(i == 0), stop=(i == NI - 1))

            # ---- per-side epilogue: transpose back, divide ----
            proj = {}
            for side in ("t", "s"):
                segT_sb = sb_s.tile([2 * Q, P], F32, tag="segTsb", name="segTsb")
                nc.scalar.copy(out=segT_sb[:], in_=segT[side][:])
                seg_ps = ps.tile([P, 2 * Q], F32, space="PSUM", tag="seg",
                                 name="seg")
                nc.tensor.transpose(out=seg_ps[:], in_=segT_sb[:],
                                    identity=ident[0:2 * Q, 0:2 * Q])
                cnt = sb_s.tile([P, Q], F32, tag="cnt")
                nc.vector.tensor_scalar(out=cnt[:], in0=seg_ps[:, Q:2 * Q],
                                        scalar1=1.0, scalar2=None,
                                        op0=mybir.AluOpType.max)
                rec = sb_s.tile([P, Q], F32, tag="rec")
                nc.vector.reciprocal(out=rec[:], in_=cnt[:])
                proj[side] = sb_s.tile([P, Q], F32, tag=f"proj{side}", name=f"proj{side}")
                nc.vector.tensor_tensor(out=proj[side][:], in0=seg_ps[:, 0:Q],
                                        in1=rec[:], op=mybir.AluOpType.mult)

            # fold bias into source projection
            proj_sb = sb_s.tile([P, Q], F32, tag="projsb")
            nc.vector.tensor_scalar(out=proj_sb[:], in0=proj["s"][:],
                                    scalar1=b_bc[:, 0:1], scalar2=None,
                                    op0=mybir.AluOpType.add)

            # ---- broadcast tgt projection to a [P, W] row: tp[p, q*128+r] = proj_t[r, q]
            msel = sb_s.tile([P, W], F32, tag="msel")
            for qb in range(Q):
                nc.vector.tensor_scalar(
                    out=msel[:, qb * P:(qb + 1) * P], in0=ident[:],
                    scalar1=proj["t"][:, qb:qb + 1], scalar2=None,
                    op0=mybir.AluOpType.mult)
            bc_sb = sb_s.tile([P, W], F32, tag="bcsb")
            for half in range(2):
                bc_ps = ps.tile([P, W // 2], F32, space="PSUM", tag="bc")
                nc.tensor.matmul(out=bc_ps[:], lhsT=ones[:],
                                 rhs=msel[:, half * (W // 2):(half + 1) * (W // 2)],
                                 start=True, stop=True)
                nc.scalar.copy(out=bc_sb[:, half * (W // 2):(half + 1) * (W // 2)],
                               in_=bc_ps[:])

            # ---- output tiles: out[j*128+p, t] = proj_s[p, j] + tp[t] ----
            out_ap = out_d[b][:].rearrange("(j p) t -> p j t", p=P)
            for j in range(Q):
                ot = sb_o.tile([P, W], F32, tag="ot")
                if b == BLOC - 1 and j % 2 == 0:
                    # tail batch: split adds across DVE and ACT
                    nc.vector.tensor_scalar(
                        out=ot[:], in0=bc_sb[:], scalar1=proj_sb[:, j:j + 1],
                        scalar2=None, op0=mybir.AluOpType.add)
                else:
                    nc.scalar.add(out=ot[:], in_=bc_sb[:], add=proj_sb[:, j:j + 1])
                nc.scalar.dma_start(out=out_ap[:, j, :], in_=ot[:])

    nc.compile()
    return nc, names


def _get_module():
    if "mod" not in _CACHE:
        _CACHE["mod"] = _build_module()
    return _CACHE["mod"]


def _run(hidden, classifier_w, classifier_b, source_word_ids, target_word_ids,
         **spmd_kwargs):
    nc, names = _get_module()
    hdtype = ml_dtypes.bfloat16 if DOT_DTYPE == "bf16" else np.float32
    hidden = np.ascontiguousarray(hidden).astype(hdtype, copy=False)
    w = np.ascontiguousarray(
        np.broadcast_to(np.asarray(classifier_w, dtype=np.float32)
                        .reshape(1, 2 * H), (P, 2 * H)).astype(hdtype))
    bias = np.ascontiguousarray(
        np.broadcast_to(np.asarray(classifier_b, dtype=np.float32)
                        .reshape(1, 1), (P, 1)))
    src = np.ascontiguousarray(source_word_ids, dtype=np.int32)
    tgt = np.ascontiguousarray(target_word_ids, dtype=np.int32)

    in_maps = []
    for c in range(NCORES):
        m = {names["w"]: w, names["b"]: bias}
        for b in range(BLOC):
            gb = c * BLOC + b
            m[names["hid"][b]] = hidden[gb]
            m[names["src"][b]] = src[gb]
            m[names["tgt"][b]] = tgt[gb]
        in_maps.append(m)

    res = run_bass_kernel_spmd(nc, in_maps, core_ids=list(range(NCORES)),
                               **spmd_kwargs)
    out = np.empty((B, W, W, 1), dtype=np.float32)
    for c in range(NCORES):
        for b in range(BLOC):
            out[c * BLOC + b, :, :, 0] = res.results[c][names["out"][b]]
    return out, res


def kernel(hidden, classifier_w, classifier_b, source_word_ids,
           target_word_ids, num_words):
    out, _ = _run(hidden, classifier_w, classifier_b, source_word_ids,
                  target_word_ids)
    return out



# revision 4
# speedup vs baseline: 1.6949x; 1.6949x over previous
"""Trainium2 Bass kernel for BinaryTokenClassificationModel (segment_reduce).

Reference semantics (B=16, L=2048, H=1024, W=1024):
    src = segment_mean(hidden, source_word_ids)   # [B,W,H]
    tgt = segment_mean(hidden, target_word_ids)   # [B,W,H]
    logits[b,s,t,0] = src[b,s]@w_s + tgt[b,t]@w_t + bias

Algebraic restructuring: the pooled [B,W,H] tensors are never
materialized.  Because the classifier is linear,
    src_proj[b,s] = segment_mean_s( hidden[b,l] @ w_s )
so per-token scalar dots are computed on the PE (hidden is staged
host-side as [H, L] so both classifier dots stream through the matmul
array in one pass), the dot rows are transposed back to token-partition
layout on the PE, the scalars are segment-reduced via one-hot bf16
matmuls (w = 128*q + r factorization), and the [W,W] output is emitted
as an outer broadcast-sum using 4x-packed bf16 tensor_scalar adds on
the DVE.  Output is stored bf16 (tolerance 2e-2; bf16 adds ~4e-3).

Sharding: data-parallel over batch — 2 examples per NeuronCore on 8
cores; the tiny classifier weights are replicated.
"""

from contextlib import ExitStack

import ml_dtypes
import numpy as np

import concourse.mybir as mybir
import concourse.tile as tile
from concourse import bacc
from concourse.bass_utils import run_bass_kernel_spmd
from concourse.masks import make_identity

P = 128          # partitions
B = 16           # full batch
NCORES = 8
BLOC = B // NCORES   # batches per core = 2
L = 2048         # tokens
H = 1024         # hidden
W = 1024         # words
Q = W // P       # 8 word chunks
HC = H // P      # 8 hidden chunks
NK = 4           # token chunks of 512 (PSUM-bank sized dot strips)
NI2 = 4          # 128-token blocks per chunk; token l = k*512 + i*128 + p
NT = NK * NI2    # 16 token tiles; tile index t16 = i*4 + k

F32 = mybir.dt.float32
BF16 = mybir.dt.bfloat16

_CACHE = {}


def _build_module():
    nc = bacc.Bacc(None, target_bir_lowering=False, debug=False)
    names = {}
    with tile.TileContext(nc) as tc, ExitStack() as ctx:
        dram = ctx.enter_context(tc.tile_pool(name="dram", bufs=1, space="DRAM"))
        sb = ctx.enter_context(tc.tile_pool(name="sb", bufs=1))
        ps = ctx.enter_context(tc.tile_pool(name="psum", bufs=1, space="PSUM"))

        hid_d = [dram.tile([H, L], BF16, kind="ExternalInput", name=f"hidT{b}")
                 for b in range(BLOC)]
        qr_d = [dram.tile([P, 4, NT], BF16, kind="ExternalInput", name=f"qr{b}")
                for b in range(BLOC)]
        w_d = dram.tile([P, HC, 2], BF16, kind="ExternalInput", name="wsb")
        b_d = dram.tile([P, 1], F32, kind="ExternalInput", name="bias")
        out_d = [dram.tile([W, W], BF16, kind="ExternalOutput", name=f"logits{b}")
                 for b in range(BLOC)]

        names["hid"] = [t.name for t in hid_d]
        names["qr"] = [t.name for t in qr_d]
        names["w"] = w_d.name
        names["b"] = b_d.name
        names["out"] = [t.name for t in out_d]

        # ---- constants ----
        wsb = sb.tile([P, HC, 2], BF16, tag="wsb")
        nc.scalar.dma_start(out=wsb[:], in_=w_d[:])
        b_bc = sb.tile([P, 1], F32, tag="bb")
        nc.scalar.dma_start(out=b_bc[:], in_=b_d[:])
        qr_t = []
        for b in range(BLOC):
            t = sb.tile([P, 4, NT], BF16, tag=f"qr{b}", name=f"qr{b}")
            nc.gpsimd.dma_start(out=t[:], in_=qr_d[b][:])
            qr_t.append(t)

        # iota_r[p, t, r] = r ; iota_q[p, t, q] = q
        iota_r = sb.tile([P, NT, P], BF16, tag="ior")
        nc.gpsimd.iota(iota_r[:], pattern=[[0, NT], [1, P]], base=0,
                       channel_multiplier=0, allow_small_or_imprecise_dtypes=True)
        iota_q = sb.tile([P, NT, Q], BF16, tag="ioq")
        nc.gpsimd.iota(iota_q[:], pattern=[[0, NT], [1, Q]], base=0,
                       channel_multiplier=0, allow_small_or_imprecise_dtypes=True)
        ident_bf = sb.tile([P, P], BF16, tag="idbf")
        make_identity(nc, ident_bf[:])
        ident_f32 = sb.tile([2 * Q, 2 * Q], F32, tag="idf32")
        make_identity(nc, ident_f32[:])
        ones_bf = sb.tile([P, P], BF16, tag="ones")
        nc.vector.memset(ones_bf[:], 1.0)

        # ---- input DMA: all hidden chunk-pairs up front, two queues ----
        # ht[b][g][p, cc, l] = hidT[(2g+cc)*128 + p, l]
        ht = []
        for b in range(BLOC):
            hid_ap = hid_d[b][:].rearrange("(c p) l -> p c l", p=P)
            row = []
            for g in range(HC // 2):
                t = sb.tile([P, 2, L], BF16, tag=f"ht{b}g{g}", name=f"ht{b}g{g}")
                eng = nc.sync if g % 2 == 0 else nc.gpsimd
                eng.dma_start(out=t[:], in_=hid_ap[:, 2 * g:2 * g + 2, :])
                row.append(t)
            ht.append(row)

        # ---- one-hot masks for both batches early (DVE warms up while PE
        # streams the dots) ----
        or_all = {}
        mdoq = {}
        for b in range(BLOC):
            for sidx, side in enumerate(("s", "t")):
                qf = qr_t[b][:, 2 * sidx + 0, :]     # [P, NT]
                rf = qr_t[b][:, 2 * sidx + 1, :]
                oa = sb.tile([P, NT, P], BF16, tag=f"or{side}{b}",
                             name=f"or{side}{b}")
                nc.vector.tensor_tensor(
                    out=oa[:], in0=iota_r[:],
                    in1=rf.to_broadcast([P, NT, P]),
                    op=mybir.AluOpType.is_equal)
                md = sb.tile([P, NT, 2 * Q], BF16, tag=f"md{side}{b}",
                             name=f"md{side}{b}")
                nc.vector.tensor_tensor(
                    out=md[:, :, Q:2 * Q], in0=iota_q[:],
                    in1=qf.to_broadcast([P, NT, Q]),
                    op=mybir.AluOpType.is_equal)
                or_all[(b, side)] = oa
                mdoq[(b, side)] = md

        for b in range(BLOC):
            # ---- dots on the PE: dps[k][2, 512] = [w_s|w_t]^T @ hidT ----
            dps = ps.tile([2, NK, 512], F32, space="PSUM", tag="dps",
                          name="dps")
            for c in range(HC):
                g, cc = divmod(c, 2)
                for k in range(NK):
                    nc.tensor.matmul(out=dps[:, k, :],
                                     lhsT=wsb[:, c, :],
                                     rhs=ht[b][g][:, cc, k * 512:(k + 1) * 512],
                                     start=(c == 0), stop=(c == HC - 1))

            # dsb[s, l] = dot_side_s(token l), l = k*512 + j
            dsb = sb.tile([2, NK, 512], BF16, tag=f"dsb{b}", name=f"dsb{b}")
            nc.scalar.copy(out=dsb[:], in_=dps[:])

            # transpose back to token-partition layout:
            # dt_ps[p, t16, s] = dot_s(t16*128 + p)
            dt_ps = ps.tile([P, NT, 2], BF16, space="PSUM", tag="dt",
                            name="dt")
            for t16 in range(NT):
                k, i2 = divmod(t16, NI2)
                nc.tensor.transpose(out=dt_ps[:, t16, :],
                                    in_=dsb[:, k, i2 * P:(i2 + 1) * P],
                                    identity=ident_bf[0:2, 0:2])
            dt_bf = sb.tile([P, NT, 2], BF16, tag=f"dt{b}", name=f"dt{b}")
            nc.vector.tensor_copy(out=dt_bf[:], in_=dt_ps[:])

            # ---- segment reduce: segT[2Q, r] over 16 token tiles ----
            sgps = ps.tile([64, P], F32, space="PSUM", tag="sg", name="sg")
            segT = {}
            for sidx, side in enumerate(("s", "t")):
                md = mdoq[(b, side)]
                nc.vector.tensor_tensor(
                    out=md[:, :, 0:Q], in0=md[:, :, Q:2 * Q],
                    in1=dt_bf[:, :, sidx].to_broadcast([P, NT, Q]),
                    op=mybir.AluOpType.mult)
                st = sgps[32 * sidx:32 * sidx + 2 * Q, :]
                for t16 in range(NT):
                    nc.tensor.matmul(out=st,
                                     lhsT=md[:, t16, :],
                                     rhs=or_all[(b, side)][:, t16, :],
                                     start=(t16 == 0), stop=(t16 == NT - 1))
                segT[side] = st

            # ---- epilogue: transpose, divide by counts ----
            proj = {}
            for side in ("t", "s"):
                st_sb = sb.tile([2 * Q, P], F32, tag=f"sgsb{side}{b}",
                                name=f"sgsb{side}{b}")
                nc.scalar.copy(out=st_sb[:], in_=segT[side])
                seg_ps = ps.tile([P, 2 * Q], F32, space="PSUM", tag="epi",
                                 name="epi")
                nc.tensor.transpose(out=seg_ps[:], in_=st_sb[:],
                                    identity=ident_f32[:])
                cnt = sb.tile([P, Q], F32, tag=f"cnt{b}")
                nc.vector.tensor_scalar(out=cnt[:], in0=seg_ps[:, Q:2 * Q],
                                        scalar1=1.0, scalar2=None,
                                        op0=mybir.AluOpType.max)
                rec = sb.tile([P, Q], F32, tag=f"rec{b}")
                nc.vector.reciprocal(out=rec[:], in_=cnt[:])
                pj = sb.tile([P, Q], F32, tag=f"pj{side}{b}",
                             name=f"pj{side}{b}")
                nc.vector.tensor_tensor(out=pj[:], in0=seg_ps[:, 0:Q],
                                        in1=rec[:], op=mybir.AluOpType.mult)
                proj[side] = pj

            # fold bias into the source projection
            pjs = sb.tile([P, Q], F32, tag=f"pjsb{b}")
            nc.vector.tensor_scalar(out=pjs[:], in0=proj["s"][:],
                                    scalar1=b_bc[:, 0:1], scalar2=None,
                                    op0=mybir.AluOpType.add)

            # ---- broadcast tgt projection to a [P, W] row ----
            msel = sb.tile([P, W], BF16, tag=f"msel{b}")
            for qb in range(Q):
                nc.vector.tensor_scalar(
                    out=msel[:, qb * P:(qb + 1) * P], in0=ident_bf[:],
                    scalar1=proj["t"][:, qb:qb + 1], scalar2=None,
                    op0=mybir.AluOpType.mult)
            bc_sb = sb.tile([P, W], BF16, tag=f"bcsb{b}")
            for half in range(2):
                bc_ps = ps.tile([P, W // 2], F32, space="PSUM",
                                tag="bc", name=f"bc{half}")
                nc.tensor.matmul(out=bc_ps[:], lhsT=ones_bf[:],
                                 rhs=msel[:, half * 512:(half + 1) * 512],
                                 start=True, stop=True)
                nc.vector.tensor_copy(
                    out=bc_sb[:, half * 512:(half + 1) * 512], in_=bc_ps[:])

            # ---- output tiles: out[j*128+p, t] = pjs[p, j] + bc[t] ----
            out_ap = out_d[b][:].rearrange("(j p) t -> p j t", p=P)
            for hh in range(2):
                osb = sb.tile([P, Q // 2, W], BF16, tag=f"o{b}h{hh}",
                              name=f"o{b}h{hh}")
                for jj in range(Q // 2):
                    j = hh * (Q // 2) + jj
                    nc.vector.tensor_scalar(
                        out=osb[:, jj, :], in0=bc_sb[:],
                        scalar1=pjs[:, j:j + 1], scalar2=None,
                        op0=mybir.AluOpType.add)
                nc.scalar.dma_start(
                    out=out_ap[:, hh * (Q // 2):(hh + 1) * (Q // 2), :],
                    in_=osb[:])

    nc.compile()
    return nc, names


def _get_module():
    if "mod" not in _CACHE:
        _CACHE["mod"] = _build_module()
    return _CACHE["mod"]


def _prep(hidden, classifier_w, classifier_b, source_word_ids, target_word_ids):
    bf16 = ml_dtypes.bfloat16
    hidT = np.ascontiguousarray(
        np.asarray(hidden, dtype=np.float32).astype(bf16).transpose(0, 2, 1))

    w = np.asarray(classifier_w, dtype=np.float32).reshape(2 * H)
    ws, wt = w[:H], w[H:]
    wsb = np.ascontiguousarray(
        np.stack([ws.reshape(HC, P), wt.reshape(HC, P)], axis=-1)
        .transpose(1, 0, 2).astype(bf16))                      # [P, HC, 2]
    bias = np.ascontiguousarray(
        np.broadcast_to(np.asarray(classifier_b, dtype=np.float32)
                        .reshape(1, 1), (P, 1)))

    def qr_plane(ids):
        a = np.asarray(ids, dtype=np.int32).reshape(NT, P)
        qf = (a >> 7).T
        rf = (a & 127).T
        return qf, rf

    qr = np.empty((B, P, 4, NT), dtype=bf16)
    for gb in range(B):
        qs, rs = qr_plane(source_word_ids[gb])
        qt, rt = qr_plane(target_word_ids[gb])
        qr[gb, :, 0, :] = qs
        qr[gb, :, 1, :] = rs
        qr[gb, :, 2, :] = qt
        qr[gb, :, 3, :] = rt
    return hidT, wsb, bias, qr


def _run(hidden, classifier_w, classifier_b, source_word_ids, target_word_ids,
         **spmd_kwargs):
    nc, names = _get_module()
    hidT, wsb, bias, qr = _prep(hidden, classifier_w, classifier_b,
                                source_word_ids, target_word_ids)

    in_maps = []
    for c in range(NCORES):
        m = {names["w"]: wsb, names["b"]: bias}
        for b in range(BLOC):
            gb = c * BLOC + b
            m[names["hid"][b]] = hidT[gb]
            m[names["qr"][b]] = qr[gb]
        in_maps.append(m)

    res = run_bass_kernel_spmd(nc, in_maps, core_ids=list(range(NCORES)),
                               **spmd_kwargs)
    out = np.empty((B, W, W, 1), dtype=np.float32)
    for c in range(NCORES):
        for b in range(BLOC):
            out[c * BLOC + b, :, :, 0] = np.asarray(
                res.results[c][names["out"][b]], dtype=np.float32)
    return out, res


def kernel(hidden, classifier_w, classifier_b, source_word_ids,
           target_word_ids, num_words):
    out, _ = _run(hidden, classifier_w, classifier_b, source_word_ids,
                  target_word_ids)
    return out
